# revision 10
# baseline (speedup 1.0000x reference)
"""DH-SNN forward pass on 8 Trainium2 NeuronCores (Bass/Tile).

Problem: nn_Dense_vanilla_40510131536152
  B=1024, T=200, I=512, H=256, O=2
  per step: d1 = x@W1.T + b1 ; mem1 = a1*mem1 + (1-a1)*d1 - spk1 ;
            spk1 = (mem1 > 1) ; mem2 = a2*mem2 + (1-a2)*(spk1@W2.T + b2) ;
            softmax/CE/argmax bookkeeping.

Strategy (data-parallel over batch, 128 rows/core):
  - Device computes ONLY the layer-1 recurrence and emits the mem1 raster
    (f32); the host thresholds it to recover the exact spike raster.
    Layout: batch on the 128 partitions, H=256 on the free dim.
    Recurrence is re-associated around the negated state
    qn = spk1 - a1*mem1 so each step is exactly 3 DVE ops + the matmul:
        mem1_t = psum_t - qn_{t-1}         (TT, reads PSUM; written straight
                                            into the f32 history buffer)
        r_t    = mem1_t * a1_bcast         (TT)
        qn_t   = (mem1_t > 1) - r_t        (scalar_tensor_tensor: the spike
                                            compare fuses with the update)
    The host recovers the exact spike raster as (mem1 > 1) from the history.
    psum_t = x_t @ W1eff.T + c1 where W1eff = (1-a1)*W1, c1 = (1-a1)*b1 are
    folded in on the host; c1 enters via a K=1 ones-row matmul.
  - Matmul precision: bf16 hi/lo 3-pass (xh@Wh + xh@Wl + xl@Wh), giving
    ~4e-6 relative error at full PE rate ("fp32" mode = exact, 4 cyc/row).
  - Layer 2 (O=2), softmax/CE/argmax/loss run on the host from the exact
    spikes - that math is tiny (B*T*2) and the per-timestep loss
    normalization needs full-batch counts anyway.

This walrus build rejects instructions with more than one sync wait; the
op/engine schedule above keeps every compute instruction at <=1 wait and a
BIR-JSON rewrite (installed below) splits any remaining multi-wait
instruction into single-wait NoOps on the same engine queue.
"""
import os
import numpy as np
import ml_dtypes
from contextlib import ExitStack

B, T, I, H, O = 1024, 200, 512, 256, 2
NCORES = 8
BS = B // NCORES            # 128 batch rows per core
KCH = I // 128              # 4 contraction chunks
WIN = 8                     # time steps per DMA window
VTH = 1.0
TIME_STEPS = 200
CODING_TIME = 10

MODE = os.environ.get("BASS_SNN_MODE", "bf16x3")   # "bf16x3" | "fp32"

_cache = {}


# ---------------------------------------------------------------- BIR patch
def _install_birpatch():
    if _cache.get("birpatch"):
        return
    import orjson
    import concourse.bass2jax as _b2j
    import concourse.bass_utils as _bu

    orig = _bu.compile_bir_kernel

    def _split_waits(bir_json: bytes) -> bytes:
        d = orjson.loads(bir_json)
        changed = False
        for fn in d.get("functions", []):
            for bb in fn.get("blocks", []):
                out = []
                for ins in bb.get("instructions", []):
                    si = ins.get("sync_info") or {}
                    waits = si.get("on_wait") or []
                    if len(waits) > 1:
                        for j, w in enumerate(waits[:-1]):
                            out.append({
                                "debug": ins.get("debug"),
                                "engine": ins["engine"],
                                "ins": [],
                                "is_reset_sema": False,
                                "name": f"{ins['name']}-ws{j}",
                                "opcode": "NoOp",
                                "outs": [],
                                "sync_info": {"on_update": [], "on_wait": [w]},
                            })
                        si["on_wait"] = [waits[-1]]
                        changed = True
                    out.append(ins)
                bb["instructions"] = out
        return orjson.dumps(d) if changed else bir_json

    def patched(bir_json: bytes, tmpdir: str, neff_name="file.neff") -> str:
        return orig(_split_waits(bir_json), tmpdir, neff_name=neff_name)

    _bu.compile_bir_kernel = patched
    _b2j.compile_bir_kernel = patched

    # allow walrus to elide redundant LDWEIGHTS (consecutive matmuls here
    # share the same stationary x chunk)
    if os.environ.get("BASS_SNN_LDWOPT", "0") == "1":
        _orig_run = _bu.run_command

        def _run_patched(cmd, **kw):
            cmd = [c.replace("--enable-ldw-opt=false", "--enable-ldw-opt=true")
                   if isinstance(c, str) else c for c in cmd]
            return _orig_run(cmd, **kw)

        _bu.run_command = _run_patched
    _cache["birpatch"] = True


# ---------------------------------------------------------------- device IR
def _build(mode):
    import concourse.bass as bass
    import concourse.tile as tile
    from concourse import mybir

    F32 = mybir.dt.float32
    BF16 = mybir.dt.bfloat16
    FP8 = mybir.dt.float8e4
    GT = mybir.AluOpType.is_gt
    MUL = mybir.AluOpType.mult
    ADD = mybir.AluOpType.add
    SUB = mybir.AluOpType.subtract

    nc = bass.Bass("TRN2", target_bir_lowering=False, debug=False)

    if mode == "bf16x3":
        # [k, i', t, hl*b] ; hl*b = hi/lo plane * 128 batch
        xin = nc.dram_tensor("xin", [KCH, 128, T, 256], BF16, kind="ExternalInput")
        wpk = nc.dram_tensor("wpk", [2, I, H], BF16, kind="ExternalInput")
        c1h_d = nc.dram_tensor("c1h", [1, H], BF16, kind="ExternalInput")
        c1l_d = nc.dram_tensor("c1l", [1, H], BF16, kind="ExternalInput")
    else:
        xin = nc.dram_tensor("xin", [KCH, 128, T, 128], F32, kind="ExternalInput")
        wpk = nc.dram_tensor("wpk", [1, I, H], F32, kind="ExternalInput")
        c1h_d = nc.dram_tensor("c1h", [1, H], F32, kind="ExternalInput")
    alb_d = nc.dram_tensor("alb", [BS, H], F32, kind="ExternalInput")
    histo = nc.dram_tensor("histo", [BS, T, H], F32, kind="ExternalOutput")

    xdt = BF16 if mode == "bf16x3" else F32
    xw = 256 if mode == "bf16x3" else 128          # free width per (step, k)

    with tile.TileContext(nc) as tc, ExitStack() as ctx:
        const = ctx.enter_context(tc.tile_pool(name="const", bufs=1))
        xpool = ctx.enter_context(tc.tile_pool(name="xp", bufs=2))
        hpool = ctx.enter_context(tc.tile_pool(name="hp", bufs=2))
        rpool = ctx.enter_context(tc.tile_pool(name="rp", bufs=2))
        qpool = ctx.enter_context(tc.tile_pool(name="qp", bufs=2))
        pspool = ctx.enter_context(tc.tile_pool(name="ps", bufs=6, space="PSUM"))

        npieces = 2 if mode == "bf16x3" else 1
        Ws = const.tile([128, npieces * KCH * H], xdt)     # piece-major: [pc][k][h]
        c1hl = const.tile([npieces, H], xdt)               # hi/lo rows of c1
        alb = const.tile([BS, H], F32)
        ones = const.tile([npieces, 128], xdt)

        nc.sync.dma_start(
            Ws[:].rearrange("p (c k h) -> p c k h", c=npieces, k=KCH),
            wpk[:].rearrange("c (k p) h -> p c k h", p=128))
        nc.sync.dma_start(c1hl[0:1, :], c1h_d[:])
        if mode == "bf16x3":
            nc.sync.dma_start(c1hl[1:2, :], c1l_d[:])
        nc.sync.dma_start(alb[:], alb_d[:])
        nc.vector.memset(ones[:], 1.0)

        def Wp(pc, k):
            off = (pc * KCH + k) * H
            return Ws[:, off:off + H]

        qprev = qpool.tile([BS, H], F32, tag="q")
        nc.gpsimd.memset(qprev[:], 0.0)

        for w in range(T // WIN):
            xt = xpool.tile([128, WIN * KCH * xw], xdt, tag="x")
            nc.sync.dma_start(
                xt[:].rearrange("p (k s f) -> p k s f", k=KCH, s=WIN),
                xin[:, :, w * WIN:(w + 1) * WIN, :].rearrange("k p t f -> p k t f"))
            hist = hpool.tile([BS, WIN * H], F32, tag="hist")
            # absorb the DMA-lane WAR wait for the recycled hist buffer
            nc.vector.memset(hist[:1, :1], 0.0)

            for s in range(WIN):
                ps = pspool.tile([BS, H], F32, tag="ps")
                for k in range(KCH):
                    base = (k * WIN + s) * xw
                    if mode == "bf16x3":
                        xh = xt[:, base:base + 128]
                        xl = xt[:, base + 128:base + 256]
                        nc.tensor.matmul(ps[:], xh, Wp(0, k), start=(k == 0), stop=False)
                        nc.tensor.matmul(ps[:], xh, Wp(1, k), start=False, stop=False)
                        nc.tensor.matmul(ps[:], xl, Wp(0, k), start=False, stop=False)
                    else:
                        xf = xt[:, base:base + 128]
                        nc.tensor.matmul(ps[:], xf, Wp(0, k), start=(k == 0), stop=False)
                # c1 enters via one K=npieces ones-rows matmul (rows sum to
                # c1h+c1l)
                nc.tensor.matmul(ps[:], ones[:], c1hl[:], start=False, stop=True)

                mem = hist[:, s * H:(s + 1) * H]
                nc.vector.tensor_tensor(mem, ps[:], qprev[:], SUB)
                r = rpool.tile([BS, H], F32, tag="r")
                nc.vector.tensor_tensor(r[:], mem, alb[:], MUL)
                q = qpool.tile([BS, H], F32, tag="q")
                nc.vector.scalar_tensor_tensor(q[:], mem, VTH, r[:], GT, SUB)
                qprev = q

            nc.sync.dma_start(
                histo[:, w * WIN:(w + 1) * WIN, :],
                hist[:].rearrange("p (s f) -> p s f", s=WIN))
    return nc


def _get_compiled(mode):
    key = ("nc", mode)
    if key not in _cache:
        _install_birpatch()
        _cache[key] = _build(mode)
    return _cache[key]


# ------------------------------------------------------------------- host
def _prep_inputs(mode, input, W1, b1, tau_m1):
    bf16 = ml_dtypes.bfloat16
    a1 = 1.0 / (1.0 + np.exp(-tau_m1.astype(np.float64)))
    a1 = a1.astype(np.float32)                                    # sigmoid
    weff = (W1.astype(np.float32) * (1.0 - a1)[:, None]).T        # [I, H]
    weff = np.ascontiguousarray(weff)
    c1 = ((1.0 - a1) * b1.astype(np.float32))[None, :]            # [1, H]
    alb = np.ascontiguousarray(np.broadcast_to(a1[None, :], (BS, H))).astype(np.float32)

    if mode == "bf16x3":
        wh = weff.astype(bf16)
        wl = (weff - wh.astype(np.float32)).astype(bf16)
        wpk = np.stack([wh, wl], 0)                               # [2, I, H]
        c1h = c1.astype(bf16)
        c1l = (c1 - c1h.astype(np.float32)).astype(bf16)
    else:
        wpk = weff[None]
        c1h = c1
        c1l = None

    per_core = []
    for c in range(NCORES):
        sh = input[c * BS:(c + 1) * BS]                            # [BS, T, I]
        y = np.ascontiguousarray(sh.transpose(2, 1, 0))            # [I, T, BS]
        y = y.reshape(KCH, 128, T, BS).astype(np.float32)
        if mode == "bf16x3":
            hi = y.astype(bf16)
            lo = (y - hi.astype(np.float32)).astype(bf16)
            xk = np.stack([hi, lo], 3).reshape(KCH, 128, T, 256)   # [k,i',t,hl*b]
            m = {"xin": xk, "wpk": wpk, "c1h": c1h, "c1l": c1l, "alb": alb}
        else:
            m = {"xin": y, "wpk": wpk, "c1h": c1h, "alb": alb}
        per_core.append(m)
    return per_core


def _postprocess(spikes, target, position, W2, b2, tau_m2):
    """spikes: [B, T, H] float32 of exactly 0/1."""
    f32 = np.float32
    a2 = (1.0 / (1.0 + np.exp(-tau_m2.astype(np.float64)))).astype(f32)   # [O]
    z = spikes.reshape(-1, H).astype(f32) @ W2.astype(f32).T              # [B*T, O]
    z = (z + b2.astype(f32)).reshape(B, T, O)
    d2 = np.empty((B, T, O), f32)
    mem2 = np.zeros((B, O), f32)
    one_m_a2 = (f32(1.0) - a2)
    for t in range(T):
        mem2 = mem2 * a2 + one_m_a2 * z[:, t]
        d2[:, t] = mem2

    tgt = target.astype(np.int64)
    thresh = (TIME_STEPS - (position.astype(np.int64) + 1) * CODING_TIME)  # [B]
    ts = np.arange(T, dtype=np.int64)
    mask = (ts[None, :] > thresh[:, None]).astype(f32)                     # [B, T]

    m = d2.max(axis=2, keepdims=True)
    e = np.exp(d2 - m, dtype=f32)
    probs = e / e.sum(axis=2, keepdims=True)
    pm = probs.max(axis=2, keepdims=True)
    pe = np.exp(probs - pm, dtype=f32)
    logp = (probs - pm) - np.log(pe.sum(axis=2, keepdims=True), dtype=f32)
    ce = -np.take_along_axis(logp, tgt[:, :, None], axis=2)[:, :, 0]       # [B, T]

    cnt = mask.sum(axis=0)                                                 # [T]
    num = (mask * ce).sum(axis=0)                                          # [T]
    per_t = np.where(cnt > 0, num / np.maximum(cnt, f32(1.0)), f32(0.0))
    loss = f32(per_t.astype(f32).sum(dtype=f32))

    pred = np.argmax(probs, axis=2)
    correct = np.int32(((pred == tgt) & (mask > 0)).sum())
    total = np.int32(cnt.sum())
    return loss, d2, correct, total


def kernel(input, target, position, W1, b1, tau_m1, W2, b2, tau_m2):
    from concourse.bass_utils import run_bass_kernel_spmd

    mode = MODE
    nc = _get_compiled(mode)
    in_maps = _prep_inputs(mode, np.asarray(input, np.float32),
                           np.asarray(W1), np.asarray(b1), np.asarray(tau_m1))
    res = run_bass_kernel_spmd(nc, in_maps, list(range(NCORES)))
    mems = np.concatenate(
        [np.asarray(r["histo"], np.float32)[None] for r in res.results], 0)
    spikes = (mems.reshape(B, T, H) > VTH).astype(np.float32)
    return _postprocess(spikes, np.asarray(target), np.asarray(position),
                        np.asarray(W2), np.asarray(b2), np.asarray(tau_m2))
